# revision 8
# baseline (speedup 1.0000x reference)
"""Clover-Wilson Dirac operator on Trainium2 (8 NeuronCores, T-sharded).

Math summary (derived + numerically verified against the reference):
- The reference's 4-leaf "clover" Q for plane (mu,nu) factorizes as
      Q(x) = W(x) + W(x+d1)^+ + W(x+d2)^+ + W(x+d3)^+
  with W(x) = [U_mu(x) U_nu(x+mu)] [U_nu(x) U_mu(x+nu)]^+,
  d1 = nu-mu, d2 = -2mu-2nu, d3 = -2nu (unit lattice vectors).
- With G = W - W^+ (anti-Hermitian), Ftil := Q - Q^+ = G(x) - G(x+d1) - G(x+d2) - G(x+d3).
- C psi + (4+m) psi = (5+m) psi + (csw/32) * sum_p (sigma_p (x) (-i Ftil_p)) psi,
  where sigma_p is block-diagonal (2x2 chiral blocks) in this basis.
- Wilson hop uses the standard spin-projection trick (2 half-spinors per direction).

Distribution: T=32 sharded 4 slices per core; U needs halo t0-2..t0+4 (7 slices),
psi needs t0-1..t0+4. All jnp.roll shifts are pushed into host-precomputed
pre-rolled planar fp16 arrays; on-device shifted reads of the intermediate G
use DRAM->DRAM affine shuffle DMAs.
"""
import numpy as np

T, Z, Y, X = 32, 24, 24, 24
NCOL, NS = 3, 4
MASS, CSW = 0.1, 1.0
PAIRS = [(0, 1), (0, 2), (0, 3), (1, 2), (1, 3), (2, 3)]
NCORES = 8
TLOC = T // NCORES          # 4 output slices per core
NSITE = Z * Y * X           # 13824
P = 128
F = NSITE // P              # 108
NWIN = 7                    # U window slices: t0-2 .. t0+4
DIAG = 5.0 + MASS           # (4+m) + clover identity
CCLOV = CSW / 32.0          # |coefficient| of sigma (x) Ftil; overall factor -i


# ----------------------------------------------------------------- tables

def _gammas():
    i = 1j
    g0 = np.array([[0, 0, 1, 0], [0, 0, 0, 1], [1, 0, 0, 0], [0, 1, 0, 0]], np.complex128)
    g1 = np.array([[0, 0, 0, i], [0, 0, i, 0], [0, -i, 0, 0], [-i, 0, 0, 0]], np.complex128)
    g2 = np.array([[0, 0, 0, -1], [0, 0, 1, 0], [0, 1, 0, 0], [-1, 0, 0, 0]], np.complex128)
    g3 = np.array([[0, 0, i, 0], [0, 0, 0, -i], [-i, 0, 0, 0], [0, i, 0, 0]], np.complex128)
    return [g0, g1, g2, g3]


def _sigma_blocks():
    """Chiral 2x2 blocks of sigma_{mu nu} = i g_mu g_nu for each plane."""
    G = _gammas()
    ups, dns = [], []
    for mu, nu in PAIRS:
        s = 1j * (G[mu] @ G[nu])
        assert np.abs(s[:2, 2:]).max() < 1e-12 and np.abs(s[2:, :2]).max() < 1e-12
        ups.append(s[:2, :2].copy())
        dns.append(s[2:, 2:].copy())
    return ups, dns


SIG_UP, SIG_DN = _sigma_blocks()

# per-plane shift deltas (t, z, y, x) for the W-factorization
def _deltas():
    out = []
    for mu, nu in PAIRS:
        e_mu = np.zeros(4, np.int64); e_mu[mu] = 1
        e_nu = np.zeros(4, np.int64); e_nu[nu] = 1
        out.append([tuple(e_nu - e_mu), tuple(-2 * e_mu - 2 * e_nu), tuple(-2 * e_nu)])
    return out


DELTAS = _deltas()

# debug toggles (affect both simulate_core and the device program)
ENABLE_CLOVER = True
ENABLE_HOP = True
DEBUG_DUMP = False

# hop projection tables: psi_h[c] = psi[c] + coef * psi[b[c]]; lower rows:
# row_{2+c} = rc[c] * h[m[c]]  (forward, i.e. (1-gamma)); backward negates
# coef and rc. Verified against gammas in _check_hop_tables().
HOP = {
    0: dict(b=(2, 3), coef=(-1, -1), m=(0, 1), rc=(-1, -1)),
    1: dict(b=(3, 2), coef=(-1j, -1j), m=(1, 0), rc=(1j, 1j)),
    2: dict(b=(3, 2), coef=(1, -1), m=(1, 0), rc=(-1, 1)),
    3: dict(b=(2, 3), coef=(-1j, 1j), m=(0, 1), rc=(1j, -1j)),
}


def _check_hop_tables():
    G = _gammas()
    for mu, t in HOP.items():
        for sgn in (+1, -1):  # +1: (1-g) fwd ; -1: (1+g) bwd
            M = np.eye(4) - sgn * G[mu]
            # build from table
            B = np.zeros((4, 4), np.complex128)
            for c in range(2):
                B[c, c] += 1
                B[c, t['b'][c]] += sgn * t['coef'][c]
            for c in range(2):
                rc = sgn * t['rc'][c]
                B[2 + c, t['m'][c]] += rc
                B[2 + c, t['b'][t['m'][c]]] += rc * sgn * t['coef'][t['m'][c]]
            assert np.abs(B - M).max() < 1e-12, (mu, sgn, B, M)


_check_hop_tables()


# ------------------------------------------------- planar layout helpers

def _to_planar_links(U):
    """U: (T,Z,Y,X,4,3,3) complex64 -> dict of fp16 planar arrays.

    Returns variants[key] = array [T, 18, NSITE] fp16 with comp c=(i*3+j)*2+r.
    Keys: ('c', d) centered; ('f', d, e) = U_d(x+e_hat) spatial e;
          ('b', d) = U_d(x - d_hat) spatial d.
    """
    Uf32 = np.ascontiguousarray(U)  # complex64
    planar = np.empty((4, T, 18, NSITE), np.float16)
    Um = Uf32.reshape(T, NSITE, 4, 9)
    for d in range(4):
        re = Um[..., d, :].real.astype(np.float16)  # (T, NSITE, 9)
        im = Um[..., d, :].imag.astype(np.float16)
        planar[d, :, 0::2, :] = re.transpose(0, 2, 1)
        planar[d, :, 1::2, :] = im.transpose(0, 2, 1)

    def roll_sites(arr, delta):  # arr [..., NSITE]; value at x+delta
        dz, dy, dx = delta
        a = arr.reshape(*arr.shape[:-1], Z, Y, X)
        if dz: a = np.roll(a, -dz, axis=-3)
        if dy: a = np.roll(a, -dy, axis=-2)
        if dx: a = np.roll(a, -dx, axis=-1)
        return a.reshape(*arr.shape[:-1], NSITE)

    variants = {}
    for d in range(4):
        variants[('c', d)] = planar[d]
    needed_f = {(0, 1), (0, 2), (0, 3), (2, 1), (3, 1), (3, 2), (1, 2), (1, 3), (2, 3)}
    for (d, e) in needed_f:
        delta = [0, 0, 0]; delta[e - 1] = 1
        variants[('f', d, e)] = roll_sites(planar[d], delta)
    for d in (1, 2, 3):
        delta = [0, 0, 0]; delta[d - 1] = -1
        variants[('b', d)] = roll_sites(planar[d], delta)
    return variants


def _to_planar_psi(psi):
    """psi: (T,Z,Y,X,4,3) complex64 -> dict: ('c',) and ('s', e, sgn) ->
    [T, 24, NSITE] fp16, comp c=(s*3+cl)*2+r."""
    pm = psi.reshape(T, NSITE, 12)
    planar = np.empty((T, 24, NSITE), np.float16)
    planar[:, 0::2, :] = pm.real.astype(np.float16).transpose(0, 2, 1)
    planar[:, 1::2, :] = pm.imag.astype(np.float16).transpose(0, 2, 1)

    def roll_sites(arr, delta):
        dz, dy, dx = delta
        a = arr.reshape(*arr.shape[:-1], Z, Y, X)
        if dz: a = np.roll(a, -dz, axis=-3)
        if dy: a = np.roll(a, -dy, axis=-2)
        if dx: a = np.roll(a, -dx, axis=-1)
        return a.reshape(*arr.shape[:-1], NSITE)

    out = {('c',): planar}
    for e in (1, 2, 3):
        for sgn in (1, -1):
            delta = [0, 0, 0]; delta[e - 1] = sgn
            out[('s', e, sgn)] = roll_sites(planar, delta)
    return out


# ------------------------------------------------------ numpy simulator
# Step-wise fp16 mirror of the device dataflow (for validation).

def _cmm16(A, B, dag_b=False):
    """A,B: [18, N] fp16 planar 3x3 complex; returns C = A @ B(^+) fp16."""
    C = np.zeros_like(A)
    for i in range(3):
        for k in range(3):
            cre = np.zeros(A.shape[-1], np.float16)
            cim = np.zeros(A.shape[-1], np.float16)
            for j in range(3):
                ar = A[(i * 3 + j) * 2]; ai = A[(i * 3 + j) * 2 + 1]
                if dag_b:
                    br = B[(k * 3 + j) * 2]; bi = -B[(k * 3 + j) * 2 + 1].astype(np.float16)
                else:
                    br = B[(j * 3 + k) * 2]; bi = B[(j * 3 + k) * 2 + 1]
                cre = (cre + (ar * br - ai * bi)).astype(np.float16)
                cim = (cim + (ar * bi + ai * br)).astype(np.float16)
            C[(i * 3 + k) * 2] = cre
            C[(i * 3 + k) * 2 + 1] = cim
    return C


def _antiherm9(Wm):
    """W planar 18 -> G = W - W^+ in 9-comp layout:
    q*2 / q*2+1 = re/im of G[i,j] for (i,j) in [(0,1),(0,2),(1,2)]; 6+d = im G[d,d]."""
    G = np.empty((9, Wm.shape[-1]), np.float16)
    offd = [(0, 1), (0, 2), (1, 2)]
    for q, (i, j) in enumerate(offd):
        G[q * 2] = (Wm[(i * 3 + j) * 2] - Wm[(j * 3 + i) * 2]).astype(np.float16)
        G[q * 2 + 1] = (Wm[(i * 3 + j) * 2 + 1] + Wm[(j * 3 + i) * 2 + 1]).astype(np.float16)
    for d in range(3):
        G[6 + d] = (Wm[(d * 3 + d) * 2 + 1] * np.float16(2.0)).astype(np.float16)
    return G


def _f9_entry(F9, i, j):
    """(re, im) pair (arrays or (None, arr)) of Ftil[i,j] from 9-comp planar."""
    offd = {(0, 1): 0, (0, 2): 1, (1, 2): 2}
    if i == j:
        return None, F9[6 + i]
    if (i, j) in offd:
        q = offd[(i, j)]
        return F9[q * 2], F9[q * 2 + 1]
    q = offd[(j, i)]
    return -F9[q * 2], F9[q * 2 + 1]  # G[i>j] = -conj(G[j,i]) -> (-re, +im)


def _roll_sites_np(a, delta):
    dz, dy, dx = delta
    a = a.reshape(*a.shape[:-1], Z, Y, X)
    if dz: a = np.roll(a, -dz, axis=-3)
    if dy: a = np.roll(a, -dy, axis=-2)
    if dx: a = np.roll(a, -dx, axis=-1)
    return a.reshape(*a.shape[:-2], -1) if False else a.reshape(*a.shape[:-4], a.shape[-4] if a.ndim > 3 else -1, NSITE) if False else a.reshape(-1, NSITE) if a.ndim == 4 else a.reshape(NSITE)


def simulate_core(link_vars, psi_vars, t0):
    """Numpy fp16 mirror. link_vars/psi_vars: full-T variant dicts.
    Returns planar out [TLOC, 24, NSITE] float32."""
    tw = [(t0 - 2 + w) % T for w in range(NWIN)]

    def LV(key, w):
        return link_vars[key][tw[w]]

    def PV(key, w):
        return psi_vars[key][tw[w]]

    # ---- phase 1: G per plane per window slice
    Gs = {}
    for p, (mu, nu) in enumerate(PAIRS):
        ws = range(0, 6) if mu == 0 else range(2, 6)
        for w in ws:
            if mu == 0:
                M1, M2 = LV(('c', 0), w), LV(('c', nu), w + 1)
                M3, M4 = LV(('c', nu), w), LV(('f', 0, nu), w)
            else:
                M1, M2 = LV(('c', mu), w), LV(('f', nu, mu), w)
                M3, M4 = LV(('c', nu), w), LV(('f', mu, nu), w)
            A = _cmm16(M1, M2)
            B = _cmm16(M3, M4)
            Wm = _cmm16(A, B, dag_b=True)
            Gs[(p, w)] = _antiherm9(Wm)

    out = np.zeros((TLOC, 24, NSITE), np.float32)
    for o in range(TLOC):
        w = o + 2
        # ---- Ftil per plane
        F9s = []
        for p in range(6):
            acc = Gs[(p, w)].copy()
            for (dt, dz, dy, dx) in DELTAS[p]:
                g = Gs[(p, w + dt)]
                gsh = g.reshape(9, Z, Y, X)
                if dz: gsh = np.roll(gsh, -dz, axis=1)
                if dy: gsh = np.roll(gsh, -dy, axis=2)
                if dx: gsh = np.roll(gsh, -dx, axis=3)
                acc = (acc - gsh.reshape(9, NSITE)).astype(np.float16)
            F9s.append(acc)

        if not ENABLE_CLOVER:
            F9s = [np.zeros((9, NSITE), np.float16) for _ in range(6)]
        # ---- B blocks (full 6x6 complex per chirality block), fp16
        Bblk = [np.zeros((6, 6, 2, NSITE), np.float16) for _ in range(2)]
        for blk, sigs in enumerate((SIG_UP, SIG_DN)):
            for p in range(6):
                sig = sigs[p]
                for a in range(2):
                    for b in range(2):
                        s = sig[a, b]
                        if abs(s) < 1e-12:
                            continue
                        cf = -1j * CCLOV * s  # complex coefficient
                        for i in range(3):
                            for j in range(3):
                                fre, fim = _f9_entry(F9s[p], i, j)
                                A_, B_ = a * 3 + i, b * 3 + j
                                # coeff*(fre + i fim): accumulate re and im
                                cr, ci = cf.real, cf.imag
                                tgt = Bblk[blk][A_, B_]
                                if fre is not None:
                                    if cr: tgt[0] = (tgt[0] + np.float16(cr) * fre).astype(np.float16)
                                    if ci: tgt[1] = (tgt[1] + np.float16(ci) * fre).astype(np.float16)
                                if cr: tgt[1] = (tgt[1] + np.float16(cr) * fim).astype(np.float16)
                                if ci: tgt[0] = (tgt[0] - np.float16(ci) * fim).astype(np.float16)
            for A_ in range(6):
                Bblk[blk][A_, A_, 0] = (Bblk[blk][A_, A_, 0] + np.float16(DIAG)).astype(np.float16)

        # ---- apply B to psi
        psi_c = PV(('c',), w)
        for blk in range(2):
            for A_ in range(6):
                s_out = (blk * 2 + A_ // 3) * 3 + (A_ % 3)  # spinor comp index s*3+cl
                accr = np.zeros(NSITE, np.float16)
                acci = np.zeros(NSITE, np.float16)
                for B_ in range(6):
                    s_in = (blk * 2 + B_ // 3) * 3 + (B_ % 3)
                    pr = psi_c[s_in * 2]; pi = psi_c[s_in * 2 + 1]
                    br = Bblk[blk][A_, B_, 0]; bi = Bblk[blk][A_, B_, 1]
                    accr = (accr + br * pr - bi * pi).astype(np.float16)
                    acci = (acci + br * pi + bi * pr).astype(np.float16)
                out[o, s_out * 2] += accr.astype(np.float32)
                out[o, s_out * 2 + 1] += acci.astype(np.float32)

        # ---- hop terms
        for mu in (range(4) if ENABLE_HOP else ()):
            tbl = HOP[mu]
            for sgn, wpsi_key, woff, ukey, udag in (
                (+1, 'f', +1, ('c', mu), False),
                (-1, 'b', -1, ('b', mu) if mu else ('c', 0), True),
            ):
                if mu == 0:
                    psv = PV(('c',), w + woff)
                else:
                    psv = PV(('s', mu, +1 if sgn > 0 else -1), w)
                uar = LV(ukey, w) if mu else LV(ukey, w + (0 if sgn > 0 else -1))
                # project: h[c] = psi[c] + sgn*coef[c]*psi[b[c]] (2 spins x 3 col)
                h = np.zeros((2, 3, 2, NSITE), np.float16)
                for c in range(2):
                    cf = sgn * tbl['coef'][c]
                    for cl in range(3):
                        pr = psv[(c * 3 + cl) * 2]; pi = psv[(c * 3 + cl) * 2 + 1]
                        qr = psv[(tbl['b'][c] * 3 + cl) * 2]; qi = psv[(tbl['b'][c] * 3 + cl) * 2 + 1]
                        if cf == 1:
                            h[c, cl, 0] = (pr + qr).astype(np.float16); h[c, cl, 1] = (pi + qi).astype(np.float16)
                        elif cf == -1:
                            h[c, cl, 0] = (pr - qr).astype(np.float16); h[c, cl, 1] = (pi - qi).astype(np.float16)
                        elif cf == 1j:
                            h[c, cl, 0] = (pr - qi).astype(np.float16); h[c, cl, 1] = (pi + qr).astype(np.float16)
                        else:  # -1j
                            h[c, cl, 0] = (pr + qi).astype(np.float16); h[c, cl, 1] = (pi - qr).astype(np.float16)
                # color mult: uh[c, i] = sum_j U[i,j] h[c, j] (or U^+ )
                uh = np.zeros((2, 3, 2, NSITE), np.float16)
                for c in range(2):
                    for i in range(3):
                        ar = np.zeros(NSITE, np.float16); ai = np.zeros(NSITE, np.float16)
                        for j in range(3):
                            if udag:
                                ur = uar[(j * 3 + i) * 2]; ui = -uar[(j * 3 + i) * 2 + 1].astype(np.float16)
                            else:
                                ur = uar[(i * 3 + j) * 2]; ui = uar[(i * 3 + j) * 2 + 1]
                            ar = (ar + ur * h[c, j, 0] - ui * h[c, j, 1]).astype(np.float16)
                            ai = (ai + ur * h[c, j, 1] + ui * h[c, j, 0]).astype(np.float16)
                        uh[c, i, 0] = ar; uh[c, i, 1] = ai
                # accumulate: rows 0,1: -1/2*uh[c]; rows 2+c': -1/2*sgn... rc
                for c in range(2):
                    for cl in range(3):
                        out[o, (c * 3 + cl) * 2] -= 0.5 * uh[c, cl, 0].astype(np.float32)
                        out[o, (c * 3 + cl) * 2 + 1] -= 0.5 * uh[c, cl, 1].astype(np.float32)
                for cp in range(2):
                    rc = sgn * tbl['rc'][cp]
                    mm = tbl['m'][cp]
                    for cl in range(3):
                        tr = uh[mm, cl, 0].astype(np.float32); ti = uh[mm, cl, 1].astype(np.float32)
                        if rc == 1:
                            out[o, ((2 + cp) * 3 + cl) * 2] -= 0.5 * tr
                            out[o, ((2 + cp) * 3 + cl) * 2 + 1] -= 0.5 * ti
                        elif rc == -1:
                            out[o, ((2 + cp) * 3 + cl) * 2] += 0.5 * tr
                            out[o, ((2 + cp) * 3 + cl) * 2 + 1] += 0.5 * ti
                        elif rc == 1j:
                            out[o, ((2 + cp) * 3 + cl) * 2] += 0.5 * ti
                            out[o, ((2 + cp) * 3 + cl) * 2 + 1] -= 0.5 * tr
                        else:  # -1j
                            out[o, ((2 + cp) * 3 + cl) * 2] -= 0.5 * ti
                            out[o, ((2 + cp) * 3 + cl) * 2 + 1] += 0.5 * tr
    return out


def simulate(psi, U):
    """Full-lattice numpy fp16 simulation -> complex64 (T,Z,Y,X,4,3)."""
    link_vars = _to_planar_links(U)
    psi_vars = _to_planar_psi(psi)
    out = np.zeros((T, 24, NSITE), np.float32)
    for core in range(NCORES):
        out[core * TLOC:(core + 1) * TLOC] = simulate_core(link_vars, psi_vars, core * TLOC)
    res = (out[:, 0::2, :] + 1j * out[:, 1::2, :]).astype(np.complex64)
    return res.transpose(0, 2, 1).reshape(T, Z, Y, X, NS, NCOL)


# =================================================================== bass

LINK_KEYS = (
    [('c', d) for d in range(4)]
    + [('f', d, e) for (d, e) in
       [(0, 1), (0, 2), (0, 3), (2, 1), (3, 1), (3, 2), (1, 2), (1, 3), (2, 3)]]
    + [('b', d) for d in (1, 2, 3)]
)
PSI_KEYS = [('c',)] + [('s', e, sgn) for e in (1, 2, 3) for sgn in (1, -1)]


def _lname(key):
    return "u_" + "_".join(str(x) for x in key).replace('-', 'm')


def _pname(key):
    return "psi_" + "_".join(str(x) for x in key).replace('-', 'm')


def _bbuild_table():
    """Per chirality block: list of (plane, A, B(<=A), tgt_im, f9comp, coef)."""
    offd = {(0, 1): 0, (0, 2): 1, (1, 2): 2}
    tables = [[], []]
    for blk, sigs in enumerate((SIG_UP, SIG_DN)):
        for p in range(6):
            sig = sigs[p]
            for a in range(2):
                for b in range(2):
                    s = sig[a, b]
                    if abs(s) < 1e-12:
                        continue
                    cf = -1j * CCLOV * s
                    for i in range(3):
                        for j in range(3):
                            A_, B_ = a * 3 + i, b * 3 + j
                            if A_ < B_:
                                continue
                            if i == j:
                                fre = None
                                fim = (6 + i, 1.0)
                            elif (i, j) in offd:
                                q = offd[(i, j)]
                                fre = (2 * q, 1.0); fim = (2 * q + 1, 1.0)
                            else:
                                q = offd[(j, i)]
                                fre = (2 * q, -1.0); fim = (2 * q + 1, 1.0)
                            cr, ci = cf.real, cf.imag
                            for tgt_im, parts in ((0, [(fre, cr), (fim, -ci)]),
                                                  (1, [(fim, cr), (fre, ci)])):
                                if A_ == B_ and tgt_im:
                                    continue
                                for src, c0 in parts:
                                    if src is None or abs(c0) < 1e-15:
                                        continue
                                    comp, s0 = src
                                    # CCLOV is folded into the host-side
                                    # pre-scaled M2 link inputs; the device
                                    # B-build uses +-1 coefficients only.
                                    cc = c0 * s0 / CCLOV
                                    assert abs(abs(cc) - 1.0) < 1e-9, cc
                                    tables[blk].append((p, A_, B_, tgt_im, comp, float(np.sign(cc))))
    # sanity: every lower-tri re comp and offdiag im comp gets >=1 write
    for blk in range(2):
        seen = {(A_, B_, t) for (_, A_, B_, t, _, _) in tables[blk]}
        for A_ in range(6):
            for B_ in range(A_ + 1):
                assert (A_, B_, 0) in seen, (blk, A_, B_)
                if A_ != B_:
                    assert (A_, B_, 1) in seen, (blk, A_, B_)
    return tables


BTABLES = _bbuild_table()


def _axis_pieces(d, L):
    """dst[i] = src[(i+d) % L] -> (dst_start, src_start, length) pieces."""
    d %= L
    if d == 0:
        return [(0, 0, L)]
    return [(0, d, L - d), (L - d, 0, d)]


def _build_device_program():
    import concourse.bacc as bacc
    import concourse.mybir as mybir
    from concourse import tile as ctile

    FP16, FP32 = mybir.dt.float16, mybir.dt.float32
    AL = mybir.AluOpType
    nc = bacc.Bacc(None, target_bir_lowering=False)

    u_in = {k: nc.declare_dram_parameter(_lname(k), [NWIN, P, 18, F], FP16, isOutput=False)
            for k in LINK_KEYS}
    HOPU_KEYS = [('c', d) for d in range(4)] + [('b', d) for d in (1, 2, 3)]
    tu_c = {d: nc.declare_dram_parameter("tu_c_%d" % d, [NWIN, P, 18, F], FP16, isOutput=False)
            for d in (1, 2, 3)}
    uh_in = {k: nc.declare_dram_parameter("uh" + _lname(k), [NWIN, P, 18, F], FP16, isOutput=False)
             for k in HOPU_KEYS}
    p_in = {k: nc.declare_dram_parameter(_pname(k), [NWIN, P, 24, F], FP16, isOutput=False)
            for k in PSI_KEYS}
    out_dram = nc.declare_dram_parameter("out", [TLOC, P, 24, F], FP32, isOutput=True)

    dbg = {}
    if DEBUG_DUMP:
        dbg['g'] = nc.declare_dram_parameter("dbg_g", [6, NWIN, 9, NSITE], FP16, isOutput=True)
        dbg['ft'] = nc.declare_dram_parameter("dbg_ft", [6, P, 9, F], FP16, isOutput=True)
        dbg['bb'] = nc.declare_dram_parameter("dbg_bb", [2, P, 72, F], FP16, isOutput=True)
        dbg['ap'] = nc.declare_dram_parameter("dbg_ap", [P, 24, F], FP16, isOutput=True)
    gps = [[nc.dram_tensor(f"gp{p}_{w}", [9, NSITE], FP16) for w in range(NWIN)]
           for p in range(6)]
    gshs = [[[nc.dram_tensor(f"gsh{p}_{k}_{o}", [9, NSITE], FP16) for o in range(TLOC)]
             for k in range(3)] for p in range(6)]

    def emit_cmatmul(pool, out_t, a_t, b_t, dag_b, eng=None, tp="", a_rsplit=False,
                     reduce_eng=None):
        """out = A @ B(^+), 3x3 complex; muls+combine on `eng` (DVE), the
        final j-sum reduction on `reduce_eng` (Pool) to offload the DVE."""
        if eng is None:
            eng = nc.vector
        if reduce_eng is None:
            reduce_eng = eng
        PT = {}
        for ra in (0, 1):
            for rb in (0, 1):
                PT[(ra, rb)] = pool.tile([P, 27, F], FP16, tag=f"{tp}mmP{ra}{rb}",
                                         name=f"{tp}mmP{ra}{rb}", bufs=1)
        # bufs=2: the Pool-side j-sum reads these while the DVE starts the
        # next cmatmul's combine into the other buffer (avoids WAR stall).
        Dre = pool.tile([P, 27, F], FP16, tag=f"{tp}mmDre", name=f"{tp}mmDre", bufs=2)
        Dim = pool.tile([P, 27, F], FP16, tag=f"{tp}mmDim", name=f"{tp}mmDim", bufs=2)
        # P[k,i,j] = A[i,j] * Bop[k,j].  b_t comps r*9+k*3+j hold:
        #   non-dag: B[j,k] (host-transposed);  dag: B[k,j] (std r-split row,col).
        for ra in (0, 1):
            if a_rsplit:
                av = a_t[:, ra * 9:(ra + 1) * 9, :]
            else:
                av = a_t[:].rearrange("p (ij r) f -> p ij r f", r=2)[:, :, ra, :]
            av = av.unsqueeze(1).broadcast_to([P, 3, 9, F])
            for rb in (0, 1):
                bsel = b_t[:, rb * 9:(rb + 1) * 9, :].rearrange(
                    "p (k j) f -> p k j f", k=3)
                bb = bsel.unsqueeze(2).broadcast_to([P, 3, 3, 3, F])
                ov = PT[(ra, rb)][:].rearrange("p (k i j) f -> p k i j f", k=3, i=3)
                eng.tensor_mul(
                    ov, av.rearrange("p k (i j) f -> p k i j f", i=3), bb)
        if dag_b:
            eng.tensor_add(Dre[:], PT[(0, 0)][:], PT[(1, 1)][:])
            eng.tensor_sub(Dim[:], PT[(1, 0)][:], PT[(0, 1)][:])
        else:
            eng.tensor_sub(Dre[:], PT[(0, 0)][:], PT[(1, 1)][:])
            eng.tensor_add(Dim[:], PT[(0, 1)][:], PT[(1, 0)][:])
        for r, Dt in ((0, Dre), (1, Dim)):
            Dv = Dt[:].rearrange("p (k i j) f -> p k i j f", k=3, i=3)
            ov = out_t[:, r * 9:(r + 1) * 9, :].rearrange(
                "p (i k) f -> p k i f", i=3)
            reduce_eng.tensor_add(ov, Dv[:, :, :, 0, :], Dv[:, :, :, 1, :])
            reduce_eng.tensor_add(ov, ov, Dv[:, :, :, 2, :])

    def emit_cmatvec(pool, uh_t, u_t, h_t, dag):
        """uh[c,i] = sum_j Utilde[i,j] h[c,j]; h/uh: [P,12,F] (c=2 cols)."""
        if dag:
            uv = u_t[:].rearrange("p (j i r) f -> p i j r f", j=3, i=3)
        else:
            uv = u_t[:].rearrange("p (i j r) f -> p i j r f", i=3, j=3)
        hv = h_t[:].rearrange("p (c cl r) f -> p c cl r f", c=2, cl=3)
        ov = uh_t[:].rearrange("p (c i r) f -> p c i r f", c=2, i=3)
        P4 = {}
        for ra in (0, 1):
            for rb in (0, 1):
                P4[(ra, rb)] = pool.tile([P, 18, F], FP16, tag=f"mvP{ra}{rb}",
                                         name=f"mvP{ra}{rb}", bufs=1)
        Dre = pool.tile([P, 18, F], FP16, tag="mvDre", name="mvDre", bufs=1)
        Dim = pool.tile([P, 18, F], FP16, tag="mvDim", name="mvDim", bufs=1)
        # both spin components in one op: [P, c=2, i=3, j=3, F]
        for (ra, rb), pt in P4.items():
            ub = uv[:, :, :, ra, :].unsqueeze(1).broadcast_to([P, 2, 3, 3, F])
            hb = hv[:, :, :, rb, :].unsqueeze(2).broadcast_to([P, 2, 3, 3, F])
            nc.vector.tensor_mul(
                pt[:].rearrange("p (c i j) f -> p c i j f", c=2, i=3), ub, hb)
        if dag:
            nc.vector.tensor_add(Dre[:], P4[(0, 0)][:], P4[(1, 1)][:])
            nc.vector.tensor_sub(Dim[:], P4[(0, 1)][:], P4[(1, 0)][:])
        else:
            nc.vector.tensor_sub(Dre[:], P4[(0, 0)][:], P4[(1, 1)][:])
            nc.vector.tensor_add(Dim[:], P4[(0, 1)][:], P4[(1, 0)][:])
        for r, Dt in ((0, Dre), (1, Dim)):
            o1 = ov[:, :, :, r, :]
            Dv = Dt[:].rearrange("p (c i j) f -> p c i j f", c=2, i=3)
            nc.vector.tensor_add(o1, Dv[:, :, :, 0, :], Dv[:, :, :, 1, :])
            nc.vector.tensor_add(o1, o1, Dv[:, :, :, 2, :])

    GPS_TRIPLES = frozenset()
    triple_i = [0]
    shuf_q = [0]
    with ctile.TileContext(nc) as tc:
        # ---------------- phase 1: G build ----------------
        with tc.tile_pool(name="lnk", bufs=2) as lnk, \
             tc.tile_pool(name="gtmp", bufs=2) as gtmp, \
             tc.tile_pool(name="gout", bufs=2) as goutp:
            for w in range(6):
                cache = {}

                def load_link(key, wi, tag, trans=False):
                    ck = (key, wi, trans)
                    if ck not in cache:
                        t = lnk.tile([P, 18, F], FP16, tag=tag, name=tag)
                        if trans and key[0] == 'c':
                            srcp = tu_c[key[1]]
                        else:
                            srcp = u_in[key]  # ('f',*) params carry transposed data
                        nc.sync.dma_start(t[:], srcp[wi])
                        cache[ck] = t
                    return cache[ck]

                for p, (mu, nu) in enumerate(PAIRS):
                    if mu != 0 and w < 2:
                        continue
                    if mu == 0:
                        M1 = load_link(('c', 0), w, "m1_" + str(p))
                        M2 = load_link(('c', nu), w + 1, "m2_" + str(p), trans=True)
                        M3 = load_link(('c', nu), w, "m3_" + str(p))
                        M4 = load_link(('f', 0, nu), w, "m4_" + str(p), trans=True)
                    else:
                        M1 = load_link(('c', mu), w, "m1_" + str(p))
                        M2 = load_link(('f', nu, mu), w, "m2_" + str(p), trans=True)
                        M3 = load_link(('c', nu), w, "m3_" + str(p))
                        M4 = load_link(('f', mu, nu), w, "m4_" + str(p), trans=True)
                    triple_i[0] += 1
                    eng = nc.vector
                    tp = ""
                    At = gtmp.tile([P, 18, F], FP16, tag=tp + "A", name=tp + "A")
                    Bt = gtmp.tile([P, 18, F], FP16, tag=tp + "B", name=tp + "B")
                    Wt = gtmp.tile([P, 18, F], FP16, tag=tp + "W", name=tp + "W")
                    emit_cmatmul(gtmp, At, M1, M2, dag_b=False, eng=eng, tp=tp,
                                 reduce_eng=nc.gpsimd)
                    emit_cmatmul(gtmp, Bt, M3, M4, dag_b=False, eng=eng, tp=tp,
                                 reduce_eng=nc.gpsimd)
                    emit_cmatmul(gtmp, Wt, At, Bt, dag_b=True, eng=eng, tp=tp,
                                 a_rsplit=True, reduce_eng=nc.gpsimd)
                    Gt = goutp.tile([P, 9, F], FP16, tag=tp + "G", name=tp + "G")
                    offd = [(0, 1), (0, 2), (1, 2)]
                    geng = nc.gpsimd
                    for q, (i, j) in enumerate(offd):
                        a_, b_ = i * 3 + j, j * 3 + i
                        geng.tensor_sub(Gt[:, 2 * q:2 * q + 1, :],
                                        Wt[:, a_:a_ + 1, :], Wt[:, b_:b_ + 1, :])
                        geng.tensor_add(Gt[:, 2 * q + 1:2 * q + 2, :],
                                        Wt[:, 9 + a_:10 + a_, :], Wt[:, 9 + b_:10 + b_, :])
                    for d in range(3):
                        c_ = 9 + d * 3 + d
                        geng.tensor_add(Gt[:, 6 + d:7 + d, :],
                                        Wt[:, c_:c_ + 1, :], Wt[:, c_:c_ + 1, :])
                    nc.scalar.dma_start(
                        gps[p][w].rearrange("c (p2 f) -> p2 c f", p2=P), Gt[:])
                    if DEBUG_DUMP:
                        nc.sync.dma_start(
                            dbg['g'][p, w].rearrange("c (p2 f) -> p2 c f", p2=P), Gt[:])

                # G shuffles whose source slice just became ready (scalar queue)
                for p in range(6):
                    if PAIRS[p][0] != 0 and w < 2:
                        continue
                    for k, (dt, dz, dy, dx) in enumerate(DELTAS[p]):
                        for o in range(TLOC):
                            if o + 2 + dt != w:
                                continue
                            src = gps[p][w].rearrange("c (z y x) -> c z y x", z=Z, y=Y)
                            dst = gshs[p][k][o].rearrange("c (z y x) -> c z y x", z=Z, y=Y)
                            for (zd, zs, zl) in _axis_pieces(dz, Z):
                                for (yd, ys, yl) in _axis_pieces(dy, Y):
                                    for (xd, xs, xl) in _axis_pieces(dx, X):
                                        q = shuf_q[0]
                                        shuf_q[0] = (q + 1) % 2
                                        qeng = (nc.scalar, nc.sync)[q]
                                        with nc.allow_non_contiguous_dma(reason="wrap"):
                                            qeng.dma_start(
                                                dst[:, zd:zd + zl, yd:yd + yl, xd:xd + xl],
                                                src[:, zs:zs + zl, ys:ys + yl, xs:xs + xl])

        # ---------------- phase 2: apply + hop ----------------
        with tc.tile_pool(name="gld", bufs=2) as gld, \
             tc.tile_pool(name="ftl", bufs=2) as ftl, \
             tc.tile_pool(name="bbl", bufs=2) as bbl, \
             tc.tile_pool(name="psl", bufs=2) as psl, \
             tc.tile_pool(name="uhp", bufs=2) as uhp, \
             tc.tile_pool(name="htm", bufs=2) as htm, \
             tc.tile_pool(name="apl", bufs=2) as apl, \
             tc.tile_pool(name="oot", bufs=2) as oot:
            for o in range(TLOC):
                w = o + 2
                # F_tilde per plane
                ftil = []
                for p in range(6):
                    g0 = gld.tile([P, 9, F], FP16, tag="g0", name="g0")
                    nc.sync.dma_start(g0[:], gps[p][w].rearrange("c (p2 f) -> p2 c f", p2=P))
                    ft = ftl.tile([P, 9, F], FP16, tag=f"ft{p}", name=f"ft{p}")
                    first = True
                    for k in range(3):
                        gk = gld.tile([P, 9, F], FP16, tag=f"g{k + 1}", name=f"g{k + 1}")
                        nc.sync.dma_start(gk[:], gshs[p][k][o].rearrange("c (p2 f) -> p2 c f", p2=P))
                        if first:
                            nc.vector.tensor_sub(ft[:], g0[:], gk[:])
                            first = False
                        else:
                            nc.vector.tensor_sub(ft[:], ft[:], gk[:])
                    if DEBUG_DUMP and o == 0:
                        nc.sync.dma_start(dbg['ft'][p], ft[:])
                    ftil.append(ft)

                # B blocks (lower-tri build + conj fill)
                bts = [bbl.tile([P, 72, F], FP16, tag=f"B{blk}", name=f"B{blk}") for blk in range(2)]
                for blk in range(2):
                    bt = bts[blk]
                    written = set()
                    for (p, A_, B_, tgt_im, comp, coef) in (BTABLES[blk] if ENABLE_CLOVER else [(p_, A_, A_, 0, 0, 0.0) for p_ in [0] for A_ in range(6)]):
                        e = (A_ * 6 + B_) * 2 + tgt_im
                        dst = bt[:, e:e + 1, :]
                        src = ftil[p][:, comp:comp + 1, :]
                        # coef is +-1 (CCLOV folded into the pre-scaled M2
                        # inputs) -> plain copy/add/sub, all 2x-or-better DVE.
                        if e not in written:
                            if coef > 0:
                                nc.vector.tensor_copy(dst, src)
                            else:
                                nc.vector.tensor_scalar_mul(dst, src, -1.0)
                            written.add(e)
                        elif coef > 0:
                            nc.vector.tensor_add(dst, dst, src)
                        else:
                            nc.vector.tensor_sub(dst, dst, src)
                    for A_ in range(6):
                        e = (A_ * 6 + A_) * 2
                        nc.vector.tensor_scalar_add(bt[:, e:e + 1, :], bt[:, e:e + 1, :], DIAG)
                        nc.vector.memzero(bt[:, e + 1:e + 2, :])
                    for A_ in range(6):
                        for B_ in range(A_ + 1, 6):
                            esrc = (B_ * 6 + A_) * 2
                            edst = (A_ * 6 + B_) * 2
                            nc.scalar.copy(bt[:, edst:edst + 1, :], bt[:, esrc:esrc + 1, :])
                            nc.scalar.mul(bt[:, edst + 1:edst + 2, :],
                                          bt[:, esrc + 1:esrc + 2, :], -1.0)

                # apply B to psi -> out tile
                psi_c = psl.tile([P, 24, F], FP16, tag="psc", name="psc")
                nc.sync.dma_start(psi_c[:], p_in[('c',)][w])
                out_t = oot.tile([P, 24, F], FP16, tag="out", name="out")
                for blk in range(2):
                    bt = bts[blk]
                    bv = bt[:].rearrange("p (a b r) f -> p a b r f", a=6, b=6)
                    pw = psi_c[:, blk * 12:(blk + 1) * 12, :].rearrange(
                        "p (b r) f -> p b r f", b=6)
                    PQ = {}
                    for rB in (0, 1):
                        for rP in (0, 1):
                            pq = apl.tile([P, 36, F], FP16, tag=f"apP{rB}{rP}",
                                          name=f"apP{rB}{rP}", bufs=1)
                            bb = bv[:, :, :, rB, :]
                            pp = pw[:, :, rP, :].unsqueeze(1).broadcast_to([P, 6, 6, F])
                            nc.vector.tensor_mul(
                                pq[:].rearrange("p (a b) f -> p a b f", a=6), bb, pp)
                            PQ[(rB, rP)] = pq
                    Cre, Cim = PQ[(0, 0)], PQ[(0, 1)]
                    nc.vector.tensor_sub(Cre[:], PQ[(0, 0)][:], PQ[(1, 1)][:])
                    nc.vector.tensor_add(Cim[:], PQ[(0, 1)][:], PQ[(1, 0)][:])
                    t6v = PQ[(1, 1)][:, 0:18, :].rearrange("p (a h) f -> p a h f", a=6)
                    ow = out_t[:, blk * 12:(blk + 1) * 12, :].rearrange(
                        "p (a r) f -> p a r f", a=6)
                    for r_t, Ct in ((0, Cre), (1, Cim)):
                        Cv = Ct[:].rearrange("p (a b) f -> p a b f", a=6)
                        nc.vector.tensor_add(t6v, Cv[:, :, 0:3, :], Cv[:, :, 3:6, :])
                        ov = ow[:, :, r_t, :]
                        nc.vector.tensor_add(ov, t6v[:, :, 0, :], t6v[:, :, 1, :])
                        nc.vector.tensor_add(ov, ov, t6v[:, :, 2, :])

                if DEBUG_DUMP and o == 0:
                    for blk in range(2):
                        nc.sync.dma_start(dbg['bb'][blk], bts[blk][:])
                    nc.sync.dma_start(dbg['ap'][:], out_t[:])

                # hop terms
                for mu in (range(4) if ENABLE_HOP else ()):
                    tbl = HOP[mu]
                    for sgn in (1, -1):
                        # psi source tile
                        psv = psl.tile([P, 24, F], FP16, tag="psv", name="psv")
                        if mu == 0:
                            nc.sync.dma_start(psv[:], p_in[('c',)][w + (1 if sgn > 0 else -1)])
                        else:
                            nc.sync.dma_start(psv[:], p_in[('s', mu, 1 if sgn > 0 else -1)][w])
                        # U tile
                        ut = uhp.tile([P, 18, F], FP16, tag="ut", name="ut")
                        if sgn > 0:
                            nc.sync.dma_start(ut[:], uh_in[('c', mu)][w])
                        elif mu == 0:
                            nc.sync.dma_start(ut[:], uh_in[('c', 0)][w - 1])
                        else:
                            nc.sync.dma_start(ut[:], uh_in[('b', mu)][w])
                        # projection -> h [P,12,F]
                        h = htm.tile([P, 12, F], FP16, tag="h", name="h")
                        pvv = psv[:].rearrange("p (s r) f -> p s r f", r=2)
                        hvv = h[:].rearrange("p (s r) f -> p s r f", r=2)
                        for c in range(2):
                            cf = sgn * tbl['coef'][c]
                            b_ = tbl['b'][c]
                            if cf == 1:
                                nc.vector.tensor_add(h[:, c * 6:(c + 1) * 6, :],
                                                     psv[:, c * 6:(c + 1) * 6, :],
                                                     psv[:, b_ * 6:(b_ + 1) * 6, :])
                            elif cf == -1:
                                nc.vector.tensor_sub(h[:, c * 6:(c + 1) * 6, :],
                                                     psv[:, c * 6:(c + 1) * 6, :],
                                                     psv[:, b_ * 6:(b_ + 1) * 6, :])
                            else:
                                hre = hvv[:, c * 3:(c + 1) * 3, 0, :]
                                him = hvv[:, c * 3:(c + 1) * 3, 1, :]
                                pre = pvv[:, c * 3:(c + 1) * 3, 0, :]
                                pim = pvv[:, c * 3:(c + 1) * 3, 1, :]
                                qre = pvv[:, b_ * 3:(b_ + 1) * 3, 0, :]
                                qim = pvv[:, b_ * 3:(b_ + 1) * 3, 1, :]
                                if cf == 1j:
                                    nc.vector.tensor_sub(hre, pre, qim)
                                    nc.vector.tensor_add(him, pim, qre)
                                else:  # -1j
                                    nc.vector.tensor_add(hre, pre, qim)
                                    nc.vector.tensor_sub(him, pim, qre)
                        # color mult
                        uh = htm.tile([P, 12, F], FP16, tag="uh", name="uh")
                        emit_cmatvec(uhp, uh, ut, h, dag=(sgn < 0))
                        # accumulate into out; uh is pre-scaled by -0.5
                        sl = out_t[:, 0:12, :]
                        nc.vector.tensor_add(sl, sl, uh[:, 0:12, :])
                        uvv = uh[:].rearrange("p (s r) f -> p s r f", r=2)
                        ovv = out_t[:].rearrange("p (s r) f -> p s r f", r=2)
                        rcs = [sgn * tbl['rc'][cp] for cp in range(2)]
                        if rcs[0] == rcs[1] and tbl['m'] == (0, 1) and rcs[0] in (1, -1):
                            sl = out_t[:, 12:24, :]
                            if rcs[0] == 1:
                                nc.vector.tensor_add(sl, sl, uh[:, 0:12, :])
                            else:
                                nc.vector.tensor_sub(sl, sl, uh[:, 0:12, :])
                            continue
                        for cp in range(2):
                            rc = rcs[cp]
                            mm = tbl['m'][cp]
                            row = 2 + cp
                            if rc in (1, -1):
                                sl = out_t[:, row * 6:(row + 1) * 6, :]
                                if rc == 1:
                                    nc.vector.tensor_add(sl, sl, uh[:, mm * 6:(mm + 1) * 6, :])
                                else:
                                    nc.vector.tensor_sub(sl, sl, uh[:, mm * 6:(mm + 1) * 6, :])
                            else:
                                s_i = rc.imag
                                o_re = ovv[:, row * 3:(row + 1) * 3, 0, :]
                                o_im = ovv[:, row * 3:(row + 1) * 3, 1, :]
                                u_re = uvv[:, mm * 3:(mm + 1) * 3, 0, :]
                                u_im = uvv[:, mm * 3:(mm + 1) * 3, 1, :]
                                if s_i > 0:
                                    nc.vector.tensor_sub(o_re, o_re, u_im)
                                    nc.vector.tensor_add(o_im, o_im, u_re)
                                else:
                                    nc.vector.tensor_add(o_re, o_re, u_im)
                                    nc.vector.tensor_sub(o_im, o_im, u_re)

                # store (fp16 -> fp32 cast via SWDGE)
                nc.gpsimd.dma_start(out_dram[o], out_t[:])

    nc.finalize()
    return nc


_PROG_CACHE = {}


def _get_program():
    if 'nc' not in _PROG_CACHE:
        _PROG_CACHE['nc'] = _build_device_program()
    return _PROG_CACHE['nc']


def _sbuf_image(a, C):
    """[T, C, NSITE] -> [T, P, C, F] contiguous."""
    return np.ascontiguousarray(a.reshape(T, C, P, F).transpose(0, 2, 1, 3))


def build_in_maps(psi, U):
    link_vars = _to_planar_links(U)
    psi_vars = _to_planar_psi(psi)
    link_imgs = {k: _sbuf_image(v, 18) for k, v in link_vars.items()}
    psi_imgs = {k: _sbuf_image(v, 24) for k, v in psi_vars.items()}
    def _trsplit(img):
        # comps (i*3+j)*2+r -> r*9 + k*3 + j holding U[j,k]
        a = img.reshape(img.shape[0], P, 3, 3, 2, F)
        return np.ascontiguousarray(a.transpose(0, 1, 4, 3, 2, 5).reshape(
            img.shape[0], P, 18, F))

    # M2 operands (exclusively used as the second factor of the A-product):
    # fold the clover coefficient CCLOV into them host-side so the device
    # B-build needs only +-1 coefficients.
    M2_SCALED_F = {('f', 2, 1), ('f', 3, 1), ('f', 3, 2)}
    sc = np.float16(CCLOV)
    in_maps = []
    for core in range(NCORES):
        t0 = core * TLOC
        tw = [(t0 - 2 + w) % T for w in range(NWIN)]
        m = {}
        for k in LINK_KEYS:
            if k[0] == 'f':
                a = _trsplit(link_imgs[k][tw])
                m[_lname(k)] = a * sc if k in M2_SCALED_F else a
            else:
                m[_lname(k)] = np.ascontiguousarray(link_imgs[k][tw])
        for d in (1, 2, 3):
            m["tu_c_%d" % d] = _trsplit(link_imgs[('c', d)][tw]) * sc
        for k in [('c', d) for d in range(4)] + [('b', d) for d in (1, 2, 3)]:
            m["uh" + _lname(k)] = np.ascontiguousarray(link_imgs[k][tw] * np.float16(-0.5))
        for k in PSI_KEYS:
            m[_pname(k)] = np.ascontiguousarray(psi_imgs[k][tw])
        in_maps.append(m)
    return in_maps


def assemble_output(results):
    out = np.empty((T, 24, NSITE), np.float32)
    for core in range(NCORES):
        r = results[core]['out']  # [TLOC, P, 24, F] fp32
        out[core * TLOC:(core + 1) * TLOC] = r.transpose(0, 2, 1, 3).reshape(TLOC, 24, NSITE)
    res = (out[:, 0::2, :] + 1j * out[:, 1::2, :]).astype(np.complex64)
    return res.transpose(0, 2, 1).reshape(T, Z, Y, X, NS, NCOL)


def kernel(psi, U):
    psi = np.asarray(psi)
    U = np.asarray(U)
    from concourse.bass_utils import run_bass_kernel_spmd
    nc = _get_program()
    in_maps = build_in_maps(psi, U)
    res = run_bass_kernel_spmd(nc, in_maps, core_ids=list(range(NCORES)))
    return assemble_output(res.results)



# revision 28
# speedup vs baseline: 1.5683x; 1.5683x over previous
"""Clover-Wilson Dirac operator on Trainium2 (8 NeuronCores, T-sharded).

Math summary (derived + numerically verified against the reference):
- The reference's 4-leaf "clover" Q for plane (mu,nu) factorizes as
      Q(x) = W(x) + W(x+d1)^+ + W(x+d2)^+ + W(x+d3)^+
  with W(x) = [U_mu(x) U_nu(x+mu)] [U_nu(x) U_mu(x+nu)]^+,
  d1 = nu-mu, d2 = -2mu-2nu, d3 = -2nu (unit lattice vectors).
- With G = W - W^+ (anti-Hermitian), Ftil := Q - Q^+ = G(x) - G(x+d1) - G(x+d2) - G(x+d3).
- C psi + (4+m) psi = (5+m) psi + (csw/32) * sum_p (sigma_p (x) (-i Ftil_p)) psi,
  where sigma_p is block-diagonal (2x2 chiral blocks) in this basis.
- Wilson hop uses the standard spin-projection trick (2 half-spinors per direction).

Distribution: T=32 sharded 4 slices per core; U needs halo t0-2..t0+4 (7 slices),
psi needs t0-1..t0+4. All jnp.roll shifts are pushed into host-precomputed
pre-rolled planar fp16 arrays; on-device shifted reads of the intermediate G
use DRAM->DRAM affine shuffle DMAs.
"""
import numpy as np

T, Z, Y, X = 32, 24, 24, 24
NCOL, NS = 3, 4
MASS, CSW = 0.1, 1.0
PAIRS = [(0, 1), (0, 2), (0, 3), (1, 2), (1, 3), (2, 3)]
NCORES = 8
TLOC = T // NCORES          # 4 output slices per core
NSITE = Z * Y * X           # 13824
P = 128
F = NSITE // P              # 108
NWIN = 7                    # U window slices: t0-2 .. t0+4
DIAG = 5.0 + MASS           # (4+m) + clover identity
CCLOV = CSW / 32.0          # |coefficient| of sigma (x) Ftil; overall factor -i


# ----------------------------------------------------------------- tables

def _gammas():
    i = 1j
    g0 = np.array([[0, 0, 1, 0], [0, 0, 0, 1], [1, 0, 0, 0], [0, 1, 0, 0]], np.complex128)
    g1 = np.array([[0, 0, 0, i], [0, 0, i, 0], [0, -i, 0, 0], [-i, 0, 0, 0]], np.complex128)
    g2 = np.array([[0, 0, 0, -1], [0, 0, 1, 0], [0, 1, 0, 0], [-1, 0, 0, 0]], np.complex128)
    g3 = np.array([[0, 0, i, 0], [0, 0, 0, -i], [-i, 0, 0, 0], [0, i, 0, 0]], np.complex128)
    return [g0, g1, g2, g3]


def _sigma_blocks():
    """Chiral 2x2 blocks of sigma_{mu nu} = i g_mu g_nu for each plane."""
    G = _gammas()
    ups, dns = [], []
    for mu, nu in PAIRS:
        s = 1j * (G[mu] @ G[nu])
        assert np.abs(s[:2, 2:]).max() < 1e-12 and np.abs(s[2:, :2]).max() < 1e-12
        ups.append(s[:2, :2].copy())
        dns.append(s[2:, 2:].copy())
    return ups, dns


SIG_UP, SIG_DN = _sigma_blocks()

# per-plane shift deltas (t, z, y, x) for the W-factorization
def _deltas():
    out = []
    for mu, nu in PAIRS:
        e_mu = np.zeros(4, np.int64); e_mu[mu] = 1
        e_nu = np.zeros(4, np.int64); e_nu[nu] = 1
        out.append([tuple(e_nu - e_mu), tuple(-2 * e_mu - 2 * e_nu), tuple(-2 * e_nu)])
    return out


DELTAS = _deltas()

# debug toggles (affect both simulate_core and the device program)
ENABLE_CLOVER = True
ENABLE_HOP = True
DEBUG_DUMP = False

# hop projection tables: psi_h[c] = psi[c] + coef * psi[b[c]]; lower rows:
# row_{2+c} = rc[c] * h[m[c]]  (forward, i.e. (1-gamma)); backward negates
# coef and rc. Verified against gammas in _check_hop_tables().
HOP = {
    0: dict(b=(2, 3), coef=(-1, -1), m=(0, 1), rc=(-1, -1)),
    1: dict(b=(3, 2), coef=(-1j, -1j), m=(1, 0), rc=(1j, 1j)),
    2: dict(b=(3, 2), coef=(1, -1), m=(1, 0), rc=(-1, 1)),
    3: dict(b=(2, 3), coef=(-1j, 1j), m=(0, 1), rc=(1j, -1j)),
}


def _check_hop_tables():
    G = _gammas()
    for mu, t in HOP.items():
        for sgn in (+1, -1):  # +1: (1-g) fwd ; -1: (1+g) bwd
            M = np.eye(4) - sgn * G[mu]
            # build from table
            B = np.zeros((4, 4), np.complex128)
            for c in range(2):
                B[c, c] += 1
                B[c, t['b'][c]] += sgn * t['coef'][c]
            for c in range(2):
                rc = sgn * t['rc'][c]
                B[2 + c, t['m'][c]] += rc
                B[2 + c, t['b'][t['m'][c]]] += rc * sgn * t['coef'][t['m'][c]]
            assert np.abs(B - M).max() < 1e-12, (mu, sgn, B, M)


_check_hop_tables()


# ------------------------------------------------- planar layout helpers

def _to_planar_links(U):
    """U: (T,Z,Y,X,4,3,3) complex64 -> dict of fp16 planar arrays.

    Returns variants[key] = array [T, 18, NSITE] fp16 with comp c=(i*3+j)*2+r.
    Keys: ('c', d) centered; ('f', d, e) = U_d(x+e_hat) spatial e;
          ('b', d) = U_d(x - d_hat) spatial d.
    """
    Uf32 = np.ascontiguousarray(U)  # complex64
    planar = np.empty((4, T, 18, NSITE), np.float16)
    Um = Uf32.reshape(T, NSITE, 4, 9)
    for d in range(4):
        re = Um[..., d, :].real.astype(np.float16)  # (T, NSITE, 9)
        im = Um[..., d, :].imag.astype(np.float16)
        planar[d, :, 0::2, :] = re.transpose(0, 2, 1)
        planar[d, :, 1::2, :] = im.transpose(0, 2, 1)

    def roll_sites(arr, delta):  # arr [..., NSITE]; value at x+delta
        dz, dy, dx = delta
        a = arr.reshape(*arr.shape[:-1], Z, Y, X)
        if dz: a = np.roll(a, -dz, axis=-3)
        if dy: a = np.roll(a, -dy, axis=-2)
        if dx: a = np.roll(a, -dx, axis=-1)
        return a.reshape(*arr.shape[:-1], NSITE)

    variants = {}
    for d in range(4):
        variants[('c', d)] = planar[d]
    needed_f = {(0, 1), (0, 2), (0, 3), (2, 1), (3, 1), (3, 2), (1, 2), (1, 3), (2, 3)}
    for (d, e) in needed_f:
        delta = [0, 0, 0]; delta[e - 1] = 1
        variants[('f', d, e)] = roll_sites(planar[d], delta)
    for d in (1, 2, 3):
        delta = [0, 0, 0]; delta[d - 1] = -1
        variants[('b', d)] = roll_sites(planar[d], delta)
    return variants


def _to_planar_psi(psi):
    """psi: (T,Z,Y,X,4,3) complex64 -> dict: ('c',) and ('s', e, sgn) ->
    [T, 24, NSITE] fp16, comp c=(s*3+cl)*2+r."""
    pm = psi.reshape(T, NSITE, 12)
    planar = np.empty((T, 24, NSITE), np.float16)
    planar[:, 0::2, :] = pm.real.astype(np.float16).transpose(0, 2, 1)
    planar[:, 1::2, :] = pm.imag.astype(np.float16).transpose(0, 2, 1)

    def roll_sites(arr, delta):
        dz, dy, dx = delta
        a = arr.reshape(*arr.shape[:-1], Z, Y, X)
        if dz: a = np.roll(a, -dz, axis=-3)
        if dy: a = np.roll(a, -dy, axis=-2)
        if dx: a = np.roll(a, -dx, axis=-1)
        return a.reshape(*arr.shape[:-1], NSITE)

    out = {('c',): planar}
    for e in (1, 2, 3):
        for sgn in (1, -1):
            delta = [0, 0, 0]; delta[e - 1] = sgn
            out[('s', e, sgn)] = roll_sites(planar, delta)
    return out


# ------------------------------------------------------ numpy simulator
# Step-wise fp16 mirror of the device dataflow (for validation).

def _cmm16(A, B, dag_b=False):
    """A,B: [18, N] fp16 planar 3x3 complex; returns C = A @ B(^+) fp16."""
    C = np.zeros_like(A)
    for i in range(3):
        for k in range(3):
            cre = np.zeros(A.shape[-1], np.float16)
            cim = np.zeros(A.shape[-1], np.float16)
            for j in range(3):
                ar = A[(i * 3 + j) * 2]; ai = A[(i * 3 + j) * 2 + 1]
                if dag_b:
                    br = B[(k * 3 + j) * 2]; bi = -B[(k * 3 + j) * 2 + 1].astype(np.float16)
                else:
                    br = B[(j * 3 + k) * 2]; bi = B[(j * 3 + k) * 2 + 1]
                cre = (cre + (ar * br - ai * bi)).astype(np.float16)
                cim = (cim + (ar * bi + ai * br)).astype(np.float16)
            C[(i * 3 + k) * 2] = cre
            C[(i * 3 + k) * 2 + 1] = cim
    return C


def _antiherm9(Wm):
    """W planar 18 -> G = W - W^+ in 9-comp layout:
    q*2 / q*2+1 = re/im of G[i,j] for (i,j) in [(0,1),(0,2),(1,2)]; 6+d = im G[d,d]."""
    G = np.empty((9, Wm.shape[-1]), np.float16)
    offd = [(0, 1), (0, 2), (1, 2)]
    for q, (i, j) in enumerate(offd):
        G[q * 2] = (Wm[(i * 3 + j) * 2] - Wm[(j * 3 + i) * 2]).astype(np.float16)
        G[q * 2 + 1] = (Wm[(i * 3 + j) * 2 + 1] + Wm[(j * 3 + i) * 2 + 1]).astype(np.float16)
    for d in range(3):
        G[6 + d] = (Wm[(d * 3 + d) * 2 + 1] * np.float16(2.0)).astype(np.float16)
    return G


def _f9_entry(F9, i, j):
    """(re, im) pair (arrays or (None, arr)) of Ftil[i,j] from 9-comp planar."""
    offd = {(0, 1): 0, (0, 2): 1, (1, 2): 2}
    if i == j:
        return None, F9[6 + i]
    if (i, j) in offd:
        q = offd[(i, j)]
        return F9[q * 2], F9[q * 2 + 1]
    q = offd[(j, i)]
    return -F9[q * 2], F9[q * 2 + 1]  # G[i>j] = -conj(G[j,i]) -> (-re, +im)


def _roll_sites_np(a, delta):
    dz, dy, dx = delta
    a = a.reshape(*a.shape[:-1], Z, Y, X)
    if dz: a = np.roll(a, -dz, axis=-3)
    if dy: a = np.roll(a, -dy, axis=-2)
    if dx: a = np.roll(a, -dx, axis=-1)
    return a.reshape(*a.shape[:-2], -1) if False else a.reshape(*a.shape[:-4], a.shape[-4] if a.ndim > 3 else -1, NSITE) if False else a.reshape(-1, NSITE) if a.ndim == 4 else a.reshape(NSITE)


def simulate_core(link_vars, psi_vars, t0):
    """Numpy fp16 mirror. link_vars/psi_vars: full-T variant dicts.
    Returns planar out [TLOC, 24, NSITE] float32."""
    tw = [(t0 - 2 + w) % T for w in range(NWIN)]

    def LV(key, w):
        return link_vars[key][tw[w]]

    def PV(key, w):
        return psi_vars[key][tw[w]]

    # ---- phase 1: G per plane per window slice
    Gs = {}
    for p, (mu, nu) in enumerate(PAIRS):
        ws = range(0, 6) if mu == 0 else range(2, 6)
        for w in ws:
            if mu == 0:
                M1, M2 = LV(('c', 0), w), LV(('c', nu), w + 1)
                M3, M4 = LV(('c', nu), w), LV(('f', 0, nu), w)
            else:
                M1, M2 = LV(('c', mu), w), LV(('f', nu, mu), w)
                M3, M4 = LV(('c', nu), w), LV(('f', mu, nu), w)
            A = _cmm16(M1, M2)
            B = _cmm16(M3, M4)
            Wm = _cmm16(A, B, dag_b=True)
            Gs[(p, w)] = _antiherm9(Wm)

    out = np.zeros((TLOC, 24, NSITE), np.float32)
    for o in range(TLOC):
        w = o + 2
        # ---- Ftil per plane
        F9s = []
        for p in range(6):
            acc = Gs[(p, w)].copy()
            for (dt, dz, dy, dx) in DELTAS[p]:
                g = Gs[(p, w + dt)]
                gsh = g.reshape(9, Z, Y, X)
                if dz: gsh = np.roll(gsh, -dz, axis=1)
                if dy: gsh = np.roll(gsh, -dy, axis=2)
                if dx: gsh = np.roll(gsh, -dx, axis=3)
                acc = (acc - gsh.reshape(9, NSITE)).astype(np.float16)
            F9s.append(acc)

        if not ENABLE_CLOVER:
            F9s = [np.zeros((9, NSITE), np.float16) for _ in range(6)]
        # ---- B blocks (full 6x6 complex per chirality block), fp16
        Bblk = [np.zeros((6, 6, 2, NSITE), np.float16) for _ in range(2)]
        for blk, sigs in enumerate((SIG_UP, SIG_DN)):
            for p in range(6):
                sig = sigs[p]
                for a in range(2):
                    for b in range(2):
                        s = sig[a, b]
                        if abs(s) < 1e-12:
                            continue
                        cf = -1j * CCLOV * s  # complex coefficient
                        for i in range(3):
                            for j in range(3):
                                fre, fim = _f9_entry(F9s[p], i, j)
                                A_, B_ = a * 3 + i, b * 3 + j
                                # coeff*(fre + i fim): accumulate re and im
                                cr, ci = cf.real, cf.imag
                                tgt = Bblk[blk][A_, B_]
                                if fre is not None:
                                    if cr: tgt[0] = (tgt[0] + np.float16(cr) * fre).astype(np.float16)
                                    if ci: tgt[1] = (tgt[1] + np.float16(ci) * fre).astype(np.float16)
                                if cr: tgt[1] = (tgt[1] + np.float16(cr) * fim).astype(np.float16)
                                if ci: tgt[0] = (tgt[0] - np.float16(ci) * fim).astype(np.float16)
            for A_ in range(6):
                Bblk[blk][A_, A_, 0] = (Bblk[blk][A_, A_, 0] + np.float16(DIAG)).astype(np.float16)

        # ---- apply B to psi
        psi_c = PV(('c',), w)
        for blk in range(2):
            for A_ in range(6):
                s_out = (blk * 2 + A_ // 3) * 3 + (A_ % 3)  # spinor comp index s*3+cl
                accr = np.zeros(NSITE, np.float16)
                acci = np.zeros(NSITE, np.float16)
                for B_ in range(6):
                    s_in = (blk * 2 + B_ // 3) * 3 + (B_ % 3)
                    pr = psi_c[s_in * 2]; pi = psi_c[s_in * 2 + 1]
                    br = Bblk[blk][A_, B_, 0]; bi = Bblk[blk][A_, B_, 1]
                    accr = (accr + br * pr - bi * pi).astype(np.float16)
                    acci = (acci + br * pi + bi * pr).astype(np.float16)
                out[o, s_out * 2] += accr.astype(np.float32)
                out[o, s_out * 2 + 1] += acci.astype(np.float32)

        # ---- hop terms
        for mu in (range(4) if ENABLE_HOP else ()):
            tbl = HOP[mu]
            for sgn, wpsi_key, woff, ukey, udag in (
                (+1, 'f', +1, ('c', mu), False),
                (-1, 'b', -1, ('b', mu) if mu else ('c', 0), True),
            ):
                if mu == 0:
                    psv = PV(('c',), w + woff)
                else:
                    psv = PV(('s', mu, +1 if sgn > 0 else -1), w)
                uar = LV(ukey, w) if mu else LV(ukey, w + (0 if sgn > 0 else -1))
                # project: h[c] = psi[c] + sgn*coef[c]*psi[b[c]] (2 spins x 3 col)
                h = np.zeros((2, 3, 2, NSITE), np.float16)
                for c in range(2):
                    cf = sgn * tbl['coef'][c]
                    for cl in range(3):
                        pr = psv[(c * 3 + cl) * 2]; pi = psv[(c * 3 + cl) * 2 + 1]
                        qr = psv[(tbl['b'][c] * 3 + cl) * 2]; qi = psv[(tbl['b'][c] * 3 + cl) * 2 + 1]
                        if cf == 1:
                            h[c, cl, 0] = (pr + qr).astype(np.float16); h[c, cl, 1] = (pi + qi).astype(np.float16)
                        elif cf == -1:
                            h[c, cl, 0] = (pr - qr).astype(np.float16); h[c, cl, 1] = (pi - qi).astype(np.float16)
                        elif cf == 1j:
                            h[c, cl, 0] = (pr - qi).astype(np.float16); h[c, cl, 1] = (pi + qr).astype(np.float16)
                        else:  # -1j
                            h[c, cl, 0] = (pr + qi).astype(np.float16); h[c, cl, 1] = (pi - qr).astype(np.float16)
                # color mult: uh[c, i] = sum_j U[i,j] h[c, j] (or U^+ )
                uh = np.zeros((2, 3, 2, NSITE), np.float16)
                for c in range(2):
                    for i in range(3):
                        ar = np.zeros(NSITE, np.float16); ai = np.zeros(NSITE, np.float16)
                        for j in range(3):
                            if udag:
                                ur = uar[(j * 3 + i) * 2]; ui = -uar[(j * 3 + i) * 2 + 1].astype(np.float16)
                            else:
                                ur = uar[(i * 3 + j) * 2]; ui = uar[(i * 3 + j) * 2 + 1]
                            ar = (ar + ur * h[c, j, 0] - ui * h[c, j, 1]).astype(np.float16)
                            ai = (ai + ur * h[c, j, 1] + ui * h[c, j, 0]).astype(np.float16)
                        uh[c, i, 0] = ar; uh[c, i, 1] = ai
                # accumulate: rows 0,1: -1/2*uh[c]; rows 2+c': -1/2*sgn... rc
                for c in range(2):
                    for cl in range(3):
                        out[o, (c * 3 + cl) * 2] -= 0.5 * uh[c, cl, 0].astype(np.float32)
                        out[o, (c * 3 + cl) * 2 + 1] -= 0.5 * uh[c, cl, 1].astype(np.float32)
                for cp in range(2):
                    rc = sgn * tbl['rc'][cp]
                    mm = tbl['m'][cp]
                    for cl in range(3):
                        tr = uh[mm, cl, 0].astype(np.float32); ti = uh[mm, cl, 1].astype(np.float32)
                        if rc == 1:
                            out[o, ((2 + cp) * 3 + cl) * 2] -= 0.5 * tr
                            out[o, ((2 + cp) * 3 + cl) * 2 + 1] -= 0.5 * ti
                        elif rc == -1:
                            out[o, ((2 + cp) * 3 + cl) * 2] += 0.5 * tr
                            out[o, ((2 + cp) * 3 + cl) * 2 + 1] += 0.5 * ti
                        elif rc == 1j:
                            out[o, ((2 + cp) * 3 + cl) * 2] += 0.5 * ti
                            out[o, ((2 + cp) * 3 + cl) * 2 + 1] -= 0.5 * tr
                        else:  # -1j
                            out[o, ((2 + cp) * 3 + cl) * 2] -= 0.5 * ti
                            out[o, ((2 + cp) * 3 + cl) * 2 + 1] += 0.5 * tr
    return out


def simulate(psi, U):
    """Full-lattice numpy fp16 simulation -> complex64 (T,Z,Y,X,4,3)."""
    link_vars = _to_planar_links(U)
    psi_vars = _to_planar_psi(psi)
    out = np.zeros((T, 24, NSITE), np.float32)
    for core in range(NCORES):
        out[core * TLOC:(core + 1) * TLOC] = simulate_core(link_vars, psi_vars, core * TLOC)
    res = (out[:, 0::2, :] + 1j * out[:, 1::2, :]).astype(np.complex64)
    return res.transpose(0, 2, 1).reshape(T, Z, Y, X, NS, NCOL)


# =================================================================== bass

LINK_KEYS = (
    [('c', d) for d in range(4)]
    + [('f', d, e) for (d, e) in
       [(0, 1), (0, 2), (0, 3), (2, 1), (3, 1), (3, 2), (1, 2), (1, 3), (2, 3)]]
    + [('b', d) for d in (1, 2, 3)]
)
PSI_KEYS = [('c',)] + [('s', e, sgn) for e in (1, 2, 3) for sgn in (1, -1)]


def _lname(key):
    return "u_" + "_".join(str(x) for x in key).replace('-', 'm')


def _pname(key):
    return "psi_" + "_".join(str(x) for x in key).replace('-', 'm')


def _bbuild_table():
    """Per chirality block: list of (plane, A, B(<=A), tgt_im, f9comp, coef)."""
    offd = {(0, 1): 0, (0, 2): 1, (1, 2): 2}
    tables = [[], []]
    for blk, sigs in enumerate((SIG_UP, SIG_DN)):
        for p in range(6):
            sig = sigs[p]
            for a in range(2):
                for b in range(2):
                    s = sig[a, b]
                    if abs(s) < 1e-12:
                        continue
                    cf = -1j * CCLOV * s
                    for i in range(3):
                        for j in range(3):
                            A_, B_ = a * 3 + i, b * 3 + j
                            if A_ < B_:
                                continue
                            if i == j:
                                fre = None
                                fim = (6 + i, 1.0)
                            elif (i, j) in offd:
                                q = offd[(i, j)]
                                fre = (2 * q, 1.0); fim = (2 * q + 1, 1.0)
                            else:
                                q = offd[(j, i)]
                                fre = (2 * q, -1.0); fim = (2 * q + 1, 1.0)
                            cr, ci = cf.real, cf.imag
                            for tgt_im, parts in ((0, [(fre, cr), (fim, -ci)]),
                                                  (1, [(fim, cr), (fre, ci)])):
                                if A_ == B_ and tgt_im:
                                    continue
                                for src, c0 in parts:
                                    if src is None or abs(c0) < 1e-15:
                                        continue
                                    comp, s0 = src
                                    # CCLOV is folded into the host-side
                                    # pre-scaled M2 link inputs; the device
                                    # B-build uses +-1 coefficients only.
                                    cc = c0 * s0 / CCLOV
                                    assert abs(abs(cc) - 1.0) < 1e-9, cc
                                    tables[blk].append((p, A_, B_, tgt_im, comp, float(np.sign(cc))))
    # sanity: every lower-tri re comp and offdiag im comp gets >=1 write
    for blk in range(2):
        seen = {(A_, B_, t) for (_, A_, B_, t, _, _) in tables[blk]}
        for A_ in range(6):
            for B_ in range(A_ + 1):
                assert (A_, B_, 0) in seen, (blk, A_, B_)
                if A_ != B_:
                    assert (A_, B_, 1) in seen, (blk, A_, B_)
    return tables


BTABLES = _bbuild_table()


def _axis_pieces(d, L):
    """dst[i] = src[(i+d) % L] -> (dst_start, src_start, length) pieces."""
    d %= L
    if d == 0:
        return [(0, 0, L)]
    return [(0, d, L - d), (L - d, 0, d)]


def _build_device_program():
    import concourse.bacc as bacc
    import concourse.mybir as mybir
    from concourse import tile as ctile

    FP16, FP32 = mybir.dt.float16, mybir.dt.float32
    AL = mybir.AluOpType
    nc = bacc.Bacc(None, target_bir_lowering=False)

    u_in = {k: nc.declare_dram_parameter(_lname(k), [NWIN, P, 18, F], FP16, isOutput=False)
            for k in LINK_KEYS}
    HOPU_KEYS = [('c', d) for d in range(4)] + [('b', d) for d in (1, 2, 3)]
    tu_c = {d: nc.declare_dram_parameter("tu_c_%d" % d, [NWIN, P, 18, F], FP16, isOutput=False)
            for d in (1, 2, 3)}
    uh_in = {k: nc.declare_dram_parameter("uh" + _lname(k), [NWIN, P, 18, F], FP16, isOutput=False)
             for k in HOPU_KEYS}
    uhT_c0 = nc.declare_dram_parameter("uhT_c_0", [NWIN, P, 18, F], FP16, isOutput=False)
    p_in = {k: nc.declare_dram_parameter(_pname(k), [NWIN, P, 24, F], FP16, isOutput=False)
            for k in PSI_KEYS}
    out_dram = nc.declare_dram_parameter("out", [TLOC, P, 24, F], FP32, isOutput=True)

    dbg = {}
    if DEBUG_DUMP:
        dbg['g'] = nc.declare_dram_parameter("dbg_g", [6, NWIN, 9, NSITE], FP16, isOutput=True)
        dbg['ft'] = nc.declare_dram_parameter("dbg_ft", [6, P, 9, F], FP16, isOutput=True)
        dbg['bb'] = nc.declare_dram_parameter("dbg_bb", [2, P, 72, F], FP16, isOutput=True)
        dbg['ap'] = nc.declare_dram_parameter("dbg_ap", [P, 24, F], FP16, isOutput=True)
    gps = [[nc.dram_tensor(f"gp{p}_{w}", [NSITE, 9], FP16) for w in range(NWIN)]
           for p in range(6)]
    gshs = [[[nc.dram_tensor(f"gsh{p}_{k}_{o}", [NSITE, 9], FP16) for o in range(TLOC)]
             for k in range(3)] for p in range(6)]

    def emit_cmatmul(pool, out_t, a_t, b_t, dag_b, eng=None, tp="", a_rsplit=False,
                     reduce_eng=None):
        """out = A @ B(^+), 3x3 complex; muls+combine on `eng` (DVE), the
        final j-sum reduction on `reduce_eng` (Pool) to offload the DVE."""
        if eng is None:
            eng = nc.vector
        if reduce_eng is None:
            reduce_eng = eng
        PT = {}
        for ra in (0, 1):
            for rb in (0, 1):
                PT[(ra, rb)] = pool.tile([P, 27, F], FP16, tag=f"{tp}mmP{ra}{rb}",
                                         name=f"{tp}mmP{ra}{rb}", bufs=1)
        Dre = pool.tile([P, 27, F], FP16, tag=f"{tp}mmDre", name=f"{tp}mmDre", bufs=1)
        Dim = pool.tile([P, 27, F], FP16, tag=f"{tp}mmDim", name=f"{tp}mmDim", bufs=1)
        # P[k,i,j] = A[i,j] * Bop[k,j].  b_t comps r*9+k*3+j hold:
        #   non-dag: B[j,k] (host-transposed);  dag: B[k,j] (std r-split row,col).
        for ra in (0, 1):
            if a_rsplit:
                av = a_t[:, ra * 9:(ra + 1) * 9, :]
            else:
                av = a_t[:].rearrange("p (ij r) f -> p ij r f", r=2)[:, :, ra, :]
            av = av.unsqueeze(1).broadcast_to([P, 3, 9, F])
            for rb in (0, 1):
                bsel = b_t[:, rb * 9:(rb + 1) * 9, :].rearrange(
                    "p (k j) f -> p k j f", k=3)
                bb = bsel.unsqueeze(2).broadcast_to([P, 3, 3, 3, F])
                ov = PT[(ra, rb)][:].rearrange("p (k i j) f -> p k i j f", k=3, i=3)
                eng.tensor_mul(
                    ov, av.rearrange("p k (i j) f -> p k i j f", i=3), bb)
        if dag_b:
            eng.tensor_add(Dre[:], PT[(0, 0)][:], PT[(1, 1)][:])
            eng.tensor_sub(Dim[:], PT[(1, 0)][:], PT[(0, 1)][:])
        else:
            eng.tensor_sub(Dre[:], PT[(0, 0)][:], PT[(1, 1)][:])
            eng.tensor_add(Dim[:], PT[(0, 1)][:], PT[(1, 0)][:])
        for r, Dt in ((0, Dre), (1, Dim)):
            Dv = Dt[:].rearrange("p (k i j) f -> p k i j f", k=3, i=3)
            ov = out_t[:, r * 9:(r + 1) * 9, :].rearrange(
                "p (i k) f -> p k i f", i=3)
            reduce_eng.tensor_add(ov, Dv[:, :, :, 0, :], Dv[:, :, :, 1, :])
            reduce_eng.tensor_add(ov, ov, Dv[:, :, :, 2, :])

    def emit_cmatvec(pool, uh_t, u_t, h_t, dag):
        """uh[c,i] = sum_j Utilde[i,j] h[c,j]; uh: [P,12,F] (c,i,r); u_t and
        h_t are r-split (r outermost) so the batched APs stay <=3 free dims."""
        # u_t is host-pre-transposed for the dag path, so the read is always
        # row-major; `dag` only flips the complex-combine signs below.
        uv = u_t[:].rearrange("p (r i j) f -> p i j r f", i=3, j=3)
        hv = h_t[:].rearrange("p (r c cl) f -> p c cl r f", c=2, cl=3)
        ov = uh_t[:].rearrange("p (c i r) f -> p c i r f", c=2, i=3)
        P4 = {}
        for ra in (0, 1):
            for rb in (0, 1):
                P4[(ra, rb)] = pool.tile([P, 18, F], FP16, tag=f"mvP{ra}{rb}",
                                         name=f"mvP{ra}{rb}", bufs=1)
        Dre = pool.tile([P, 18, F], FP16, tag="mvDre", name="mvDre", bufs=1)
        Dim = pool.tile([P, 18, F], FP16, tag="mvDim", name="mvDim", bufs=1)
        # both spin components in one op: [P, c=2, i=3, j=3, F]
        for (ra, rb), pt in P4.items():
            ub = uv[:, :, :, ra, :].unsqueeze(1).broadcast_to([P, 2, 3, 3, F])
            hb = hv[:, :, :, rb, :].unsqueeze(2).broadcast_to([P, 2, 3, 3, F])
            nc.vector.tensor_mul(
                pt[:].rearrange("p (c i j) f -> p c i j f", c=2, i=3), ub, hb)
        if dag:
            nc.vector.tensor_add(Dre[:], P4[(0, 0)][:], P4[(1, 1)][:])
            nc.vector.tensor_sub(Dim[:], P4[(0, 1)][:], P4[(1, 0)][:])
        else:
            nc.vector.tensor_sub(Dre[:], P4[(0, 0)][:], P4[(1, 1)][:])
            nc.vector.tensor_add(Dim[:], P4[(0, 1)][:], P4[(1, 0)][:])
        for r, Dt in ((0, Dre), (1, Dim)):
            o1 = ov[:, :, :, r, :]
            Dv = Dt[:].rearrange("p (c i j) f -> p c i j f", c=2, i=3)
            nc.vector.tensor_add(o1, Dv[:, :, :, 0, :], Dv[:, :, :, 1, :])
            nc.vector.tensor_add(o1, o1, Dv[:, :, :, 2, :])

    GPS_TRIPLES = frozenset()
    triple_i = [0]
    shuf_q = [0]
    with ctile.TileContext(nc) as tc:
        # ---------------- phase 1: G build ----------------
        with tc.tile_pool(name="lnk", bufs=2) as lnk, \
             tc.tile_pool(name="gtmp", bufs=2) as gtmp, \
             tc.tile_pool(name="gout", bufs=2) as goutp:
            for w in range(6):
                cache = {}

                def load_link(key, wi, tag, trans=False):
                    ck = (key, wi, trans)
                    if ck not in cache:
                        t = lnk.tile([P, 18, F], FP16, tag=tag, name=tag)
                        if trans and key[0] == 'c':
                            srcp = tu_c[key[1]]
                        else:
                            srcp = u_in[key]  # ('f',*) params carry transposed data
                        nc.scalar.dma_start(t[:], srcp[wi])
                        cache[ck] = t
                    return cache[ck]

                for p, (mu, nu) in enumerate(PAIRS):
                    if mu != 0 and w < 2:
                        continue
                    if mu == 0:
                        M1 = load_link(('c', 0), w, "m1_" + str(p))
                        M2 = load_link(('c', nu), w + 1, "m2_" + str(p), trans=True)
                        M3 = load_link(('c', nu), w, "m3_" + str(p))
                        M4 = load_link(('f', 0, nu), w, "m4_" + str(p), trans=True)
                    else:
                        M1 = load_link(('c', mu), w, "m1_" + str(p))
                        M2 = load_link(('f', nu, mu), w, "m2_" + str(p), trans=True)
                        M3 = load_link(('c', nu), w, "m3_" + str(p))
                        M4 = load_link(('f', mu, nu), w, "m4_" + str(p), trans=True)
                    triple_i[0] += 1
                    eng = nc.vector
                    tp = ""
                    At = gtmp.tile([P, 18, F], FP16, tag=tp + "A", name=tp + "A")
                    Bt = gtmp.tile([P, 18, F], FP16, tag=tp + "B", name=tp + "B")
                    Wt = gtmp.tile([P, 18, F], FP16, tag=tp + "W", name=tp + "W")
                    emit_cmatmul(gtmp, At, M1, M2, dag_b=False, eng=eng, tp=tp)
                    emit_cmatmul(gtmp, Bt, M3, M4, dag_b=False, eng=eng, tp=tp)
                    emit_cmatmul(gtmp, Wt, At, Bt, dag_b=True, eng=eng, tp=tp,
                                 a_rsplit=True)
                    # G tile is comp-INNERMOST [P, F, 9] so the DRAM image is
                    # [site, 9]: spatial-roll shuffle DMAs then move all 9
                    # comps per site run (~10x fewer, ~9x larger descriptors).
                    Gt = goutp.tile([P, F, 9], FP16, tag=tp + "G", name=tp + "G")
                    gv = Gt[:].rearrange("p f c -> p c f")
                    offd = [(0, 1), (0, 2), (1, 2)]
                    for q, (i, j) in enumerate(offd):
                        a_, b_ = i * 3 + j, j * 3 + i
                        eng.tensor_sub(gv[:, 2 * q:2 * q + 1, :],
                                       Wt[:, a_:a_ + 1, :], Wt[:, b_:b_ + 1, :])
                        eng.tensor_add(gv[:, 2 * q + 1:2 * q + 2, :],
                                       Wt[:, 9 + a_:10 + a_, :], Wt[:, 9 + b_:10 + b_, :])
                    for d in range(3):
                        c_ = 9 + d * 3 + d
                        nc.vector.tensor_scalar_mul(gv[:, 6 + d:7 + d, :],
                                                    Wt[:, c_:c_ + 1, :], 2.0)
                    nc.sync.dma_start(
                        gps[p][w].rearrange("(p2 f) c -> p2 f c", p2=P), Gt[:])

                # G shuffles whose source slice just became ready (scalar queue)
                for p in range(6):
                    if PAIRS[p][0] != 0 and w < 2:
                        continue
                    for k, (dt, dz, dy, dx) in enumerate(DELTAS[p]):
                        for o in range(TLOC):
                            if o + 2 + dt != w:
                                continue
                            src = gps[p][w].rearrange("(z y x) c -> z y x c", z=Z, y=Y)
                            dst = gshs[p][k][o].rearrange("(z y x) c -> z y x c", z=Z, y=Y)
                            for (zd, zs, zl) in _axis_pieces(dz, Z):
                                for (yd, ys, yl) in _axis_pieces(dy, Y):
                                    for (xd, xs, xl) in _axis_pieces(dx, X):
                                        # shuffle bursts ride the SP queue;
                                        # loads dispatch from Act so they are
                                        # never stuck behind a shuffle burst
                                        qeng = nc.sync
                                        with nc.allow_non_contiguous_dma(reason="wrap"):
                                            qeng.dma_start(
                                                dst[zd:zd + zl, yd:yd + yl, xd:xd + xl, :],
                                                src[zs:zs + zl, ys:ys + yl, xs:xs + xl, :])

        # ---------------- phase 2: apply + hop ----------------
        with tc.tile_pool(name="gld", bufs=2) as gld, \
             tc.tile_pool(name="ftl", bufs=2) as ftl, \
             tc.tile_pool(name="bbl", bufs=2) as bbl, \
             tc.tile_pool(name="psl", bufs=2) as psl, \
             tc.tile_pool(name="uhp", bufs=2) as uhp, \
             tc.tile_pool(name="htm", bufs=2) as htm, \
             tc.tile_pool(name="apl", bufs=2) as apl, \
             tc.tile_pool(name="oot", bufs=2) as oot:
            for o in range(TLOC):
                w = o + 2
                # F_tilde per plane: loads + subs in comp-innermost [P, F, 9]
                # (contiguous, 2x DVE), then one Act-engine transposing copy
                # back to comp-major [P, 9, F] for the B-build.
                ftil = []
                for p in range(6):
                    g0 = gld.tile([P, F, 9], FP16, tag="g0", name="g0")
                    nc.scalar.dma_start(g0[:], gps[p][w].rearrange("(p2 f) c -> p2 f c", p2=P))
                    ft = ftl.tile([P, F, 9], FP16, tag=f"ft{p}", name=f"ft{p}")
                    first = True
                    for k in range(3):
                        gk = gld.tile([P, F, 9], FP16, tag=f"g{k + 1}", name=f"g{k + 1}")
                        nc.scalar.dma_start(gk[:], gshs[p][k][o].rearrange("(p2 f) c -> p2 f c", p2=P))
                        if first:
                            nc.vector.tensor_sub(ft[:], g0[:], gk[:])
                            first = False
                        else:
                            nc.vector.tensor_sub(ft[:], ft[:], gk[:])
                    ftt = ftl.tile([P, 9, F], FP16, tag=f"ftt{p}", name=f"ftt{p}",
                                   bufs=1)
                    nc.scalar.copy(ftt[:], ft[:].rearrange("p f c -> p c f"))
                    ftil.append(ftt)

                # B blocks (lower-tri build + conj fill)
                bts = [bbl.tile([P, 72, F], FP16, tag=f"B{blk}", name=f"B{blk}",
                                bufs=1) for blk in range(2)]
                for blk in range(2):
                    bt = bts[blk]
                    written = set()
                    for (p, A_, B_, tgt_im, comp, coef) in (BTABLES[blk] if ENABLE_CLOVER else [(p_, A_, A_, 0, 0, 0.0) for p_ in [0] for A_ in range(6)]):
                        e = (A_ * 6 + B_) * 2 + tgt_im
                        dst = bt[:, e:e + 1, :]
                        src = ftil[p][:, comp:comp + 1, :]
                        # coef is +-1 (CCLOV folded into the pre-scaled M2
                        # inputs) -> plain copy/add/sub, all 2x-or-better DVE.
                        if e not in written:
                            if coef > 0:
                                nc.vector.tensor_copy(dst, src)
                            else:
                                nc.vector.tensor_scalar_mul(dst, src, -1.0)
                            written.add(e)
                        elif coef > 0:
                            nc.vector.tensor_add(dst, dst, src)
                        else:
                            nc.vector.tensor_sub(dst, dst, src)
                    for A_ in range(6):
                        e = (A_ * 6 + A_) * 2
                        nc.vector.tensor_scalar_add(bt[:, e:e + 1, :], bt[:, e:e + 1, :], DIAG)
                        nc.vector.memzero(bt[:, e + 1:e + 2, :])
                    for A_ in range(6):
                        for B_ in range(A_ + 1, 6):
                            esrc = (B_ * 6 + A_) * 2
                            edst = (A_ * 6 + B_) * 2
                            nc.scalar.copy(bt[:, edst:edst + 1, :], bt[:, esrc:esrc + 1, :])
                            nc.scalar.mul(bt[:, edst + 1:edst + 2, :],
                                          bt[:, esrc + 1:esrc + 2, :], -1.0)

                # apply B to psi -> out tile
                psi_c = psl.tile([P, 24, F], FP16, tag="psc", name="psc")
                nc.scalar.dma_start(psi_c[:], p_in[('c',)][w])
                out_t = oot.tile([P, 24, F], FP16, tag="out", name="out")
                for blk in range(2):
                    bt = bts[blk]
                    bv = bt[:].rearrange("p (a b r) f -> p a b r f", a=6, b=6)
                    pw = psi_c[:, blk * 12:(blk + 1) * 12, :].rearrange(
                        "p (b r) f -> p b r f", b=6)
                    PQ = {}
                    for rB in (0, 1):
                        for rP in (0, 1):
                            pq = apl.tile([P, 36, F], FP16, tag=f"apP{rB}{rP}",
                                          name=f"apP{rB}{rP}", bufs=1)
                            # (bufs=1: produced and consumed on DVE only)
                            bb = bv[:, :, :, rB, :]
                            pp = pw[:, :, rP, :].unsqueeze(1).broadcast_to([P, 6, 6, F])
                            nc.vector.tensor_mul(
                                pq[:].rearrange("p (a b) f -> p a b f", a=6), bb, pp)
                            PQ[(rB, rP)] = pq
                    Cre, Cim = PQ[(0, 0)], PQ[(0, 1)]
                    nc.vector.tensor_sub(Cre[:], PQ[(0, 0)][:], PQ[(1, 1)][:])
                    nc.vector.tensor_add(Cim[:], PQ[(0, 1)][:], PQ[(1, 0)][:])
                    t6v = PQ[(1, 1)][:, 0:18, :].rearrange("p (a h) f -> p a h f", a=6)
                    ow = out_t[:, blk * 12:(blk + 1) * 12, :].rearrange(
                        "p (a r) f -> p a r f", a=6)
                    for r_t, Ct in ((0, Cre), (1, Cim)):
                        Cv = Ct[:].rearrange("p (a b) f -> p a b f", a=6)
                        nc.vector.tensor_add(t6v, Cv[:, :, 0:3, :], Cv[:, :, 3:6, :])
                        ov = ow[:, :, r_t, :]
                        nc.vector.tensor_add(ov, t6v[:, :, 0, :], t6v[:, :, 1, :])
                        nc.vector.tensor_add(ov, ov, t6v[:, :, 2, :])

                if DEBUG_DUMP and o == 0:
                    for blk in range(2):
                        nc.sync.dma_start(dbg['bb'][blk], bts[blk][:])
                    nc.sync.dma_start(dbg['ap'][:], out_t[:])

                # hop terms
                for mu in (range(4) if ENABLE_HOP else ()):
                    tbl = HOP[mu]
                    for sgn in (1, -1):
                        # psi source tile
                        psv = psl.tile([P, 24, F], FP16, tag="psv", name="psv")
                        if mu == 0:
                            nc.scalar.dma_start(psv[:], p_in[('c',)][w + (1 if sgn > 0 else -1)])
                        else:
                            nc.scalar.dma_start(psv[:], p_in[('s', mu, 1 if sgn > 0 else -1)][w])
                        # U tile
                        ut = uhp.tile([P, 18, F], FP16, tag="ut", name="ut")
                        if sgn > 0:
                            nc.scalar.dma_start(ut[:], uh_in[('c', mu)][w])
                        elif mu == 0:
                            nc.scalar.dma_start(ut[:], uhT_c0[w - 1])
                        else:
                            nc.scalar.dma_start(ut[:], uh_in[('b', mu)][w])
                        # projection -> h [P,12,F], r-split (re plane, im plane)
                        h = htm.tile([P, 12, F], FP16, tag="h", name="h")
                        pvv = psv[:].rearrange("p (s r) f -> p s r f", r=2)
                        for c in range(2):
                            cf = sgn * tbl['coef'][c]
                            b_ = tbl['b'][c]
                            hre = h[:, c * 3:(c + 1) * 3, :]
                            him = h[:, 6 + c * 3:6 + (c + 1) * 3, :]
                            pre = pvv[:, c * 3:(c + 1) * 3, 0, :]
                            pim = pvv[:, c * 3:(c + 1) * 3, 1, :]
                            qre = pvv[:, b_ * 3:(b_ + 1) * 3, 0, :]
                            qim = pvv[:, b_ * 3:(b_ + 1) * 3, 1, :]
                            if cf == 1:
                                nc.vector.tensor_add(hre, pre, qre)
                                nc.vector.tensor_add(him, pim, qim)
                            elif cf == -1:
                                nc.vector.tensor_sub(hre, pre, qre)
                                nc.vector.tensor_sub(him, pim, qim)
                            elif cf == 1j:
                                nc.vector.tensor_sub(hre, pre, qim)
                                nc.vector.tensor_add(him, pim, qre)
                            else:  # -1j
                                nc.vector.tensor_add(hre, pre, qim)
                                nc.vector.tensor_sub(him, pim, qre)
                        # color mult
                        uh = htm.tile([P, 12, F], FP16, tag="uh", name="uh")
                        emit_cmatvec(uhp, uh, ut, h, dag=(sgn < 0))
                        # accumulate into out; uh is pre-scaled by -0.5
                        sl = out_t[:, 0:12, :]
                        nc.vector.tensor_add(sl, sl, uh[:, 0:12, :])
                        uvv = uh[:].rearrange("p (s r) f -> p s r f", r=2)
                        ovv = out_t[:].rearrange("p (s r) f -> p s r f", r=2)
                        rcs = [sgn * tbl['rc'][cp] for cp in range(2)]
                        if rcs[0] == rcs[1] and tbl['m'] == (0, 1) and rcs[0] in (1, -1):
                            sl = out_t[:, 12:24, :]
                            if rcs[0] == 1:
                                nc.vector.tensor_add(sl, sl, uh[:, 0:12, :])
                            else:
                                nc.vector.tensor_sub(sl, sl, uh[:, 0:12, :])
                            continue
                        for cp in range(2):
                            rc = rcs[cp]
                            mm = tbl['m'][cp]
                            row = 2 + cp
                            if rc in (1, -1):
                                sl = out_t[:, row * 6:(row + 1) * 6, :]
                                if rc == 1:
                                    nc.vector.tensor_add(sl, sl, uh[:, mm * 6:(mm + 1) * 6, :])
                                else:
                                    nc.vector.tensor_sub(sl, sl, uh[:, mm * 6:(mm + 1) * 6, :])
                            else:
                                s_i = rc.imag
                                o_re = ovv[:, row * 3:(row + 1) * 3, 0, :]
                                o_im = ovv[:, row * 3:(row + 1) * 3, 1, :]
                                u_re = uvv[:, mm * 3:(mm + 1) * 3, 0, :]
                                u_im = uvv[:, mm * 3:(mm + 1) * 3, 1, :]
                                if s_i > 0:
                                    nc.vector.tensor_sub(o_re, o_re, u_im)
                                    nc.vector.tensor_add(o_im, o_im, u_re)
                                else:
                                    nc.vector.tensor_add(o_re, o_re, u_im)
                                    nc.vector.tensor_sub(o_im, o_im, u_re)

                # store (fp16 -> fp32 cast via SWDGE)
                nc.gpsimd.dma_start(out_dram[o], out_t[:])

    nc.finalize()
    return nc


_PROG_CACHE = {}


def _get_program():
    if 'nc' not in _PROG_CACHE:
        _PROG_CACHE['nc'] = _build_device_program()
    return _PROG_CACHE['nc']


def _sbuf_image(a, C):
    """[T, C, NSITE] -> [T, P, C, F] contiguous."""
    return np.ascontiguousarray(a.reshape(T, C, P, F).transpose(0, 2, 1, 3))


def build_in_maps(psi, U):
    link_vars = _to_planar_links(U)
    psi_vars = _to_planar_psi(psi)
    link_imgs = {k: _sbuf_image(v, 18) for k, v in link_vars.items()}
    psi_imgs = {k: _sbuf_image(v, 24) for k, v in psi_vars.items()}
    def _trsplit(img):
        # comps (i*3+j)*2+r -> r*9 + k*3 + j holding U[j,k]
        a = img.reshape(img.shape[0], P, 3, 3, 2, F)
        return np.ascontiguousarray(a.transpose(0, 1, 4, 3, 2, 5).reshape(
            img.shape[0], P, 18, F))

    # M2 operands (exclusively used as the second factor of the A-product):
    # fold the clover coefficient CCLOV into them host-side so the device
    # B-build needs only +-1 coefficients.
    M2_SCALED_F = {('f', 2, 1), ('f', 3, 1), ('f', 3, 2)}
    sc = np.float16(CCLOV)
    in_maps = []
    for core in range(NCORES):
        t0 = core * TLOC
        tw = [(t0 - 2 + w) % T for w in range(NWIN)]
        m = {}
        for k in LINK_KEYS:
            if k[0] == 'f':
                a = _trsplit(link_imgs[k][tw])
                m[_lname(k)] = a * sc if k in M2_SCALED_F else a
            else:
                m[_lname(k)] = np.ascontiguousarray(link_imgs[k][tw])
        for d in (1, 2, 3):
            m["tu_c_%d" % d] = _trsplit(link_imgs[('c', d)][tw]) * sc
        # hop-U arrays are r-split (comp = r*9 + row*3 + col) so the batched
        # matvec muls have mergeable (<=3 free dim) access patterns; arrays
        # consumed as U^T (dag path) are pre-transposed host-side.
        def _rsplit_hop(k, transpose):
            a = (link_imgs[k][tw] * np.float16(-0.5)).reshape(NWIN, P, 3, 3, 2, F)
            if transpose:
                a = a.transpose(0, 1, 4, 3, 2, 5)
            else:
                a = a.transpose(0, 1, 4, 2, 3, 5)
            return np.ascontiguousarray(a.reshape(NWIN, P, 18, F))

        for k in [('c', d) for d in range(4)]:
            m["uh" + _lname(k)] = _rsplit_hop(k, False)
        for k in [('b', d) for d in (1, 2, 3)]:
            m["uh" + _lname(k)] = _rsplit_hop(k, True)
        m["uhT_c_0"] = _rsplit_hop(('c', 0), True)
        for k in PSI_KEYS:
            m[_pname(k)] = np.ascontiguousarray(psi_imgs[k][tw])
        in_maps.append(m)
    return in_maps


def assemble_output(results):
    out = np.empty((T, 24, NSITE), np.float32)
    for core in range(NCORES):
        r = results[core]['out']  # [TLOC, P, 24, F] fp32
        out[core * TLOC:(core + 1) * TLOC] = r.transpose(0, 2, 1, 3).reshape(TLOC, 24, NSITE)
    res = (out[:, 0::2, :] + 1j * out[:, 1::2, :]).astype(np.complex64)
    return res.transpose(0, 2, 1).reshape(T, Z, Y, X, NS, NCOL)


def kernel(psi, U):
    psi = np.asarray(psi)
    U = np.asarray(U)
    from concourse.bass_utils import run_bass_kernel_spmd
    nc = _get_program()
    in_maps = build_in_maps(psi, U)
    res = run_bass_kernel_spmd(nc, in_maps, core_ids=list(range(NCORES)))
    return assemble_output(res.results)

